# revision 3
# baseline (speedup 1.0000x reference)
"""Depthwise cross-correlation (DepthwiseRPN) on 8 TRN2 NeuronCores.

Reference op:
  z_f: [B=128, C=256, 7, 7]   per-(b,c) kernels
  x_f: [B=128, C=256, 31, 31] search windows
  out: [B=128, C=256, 25, 25] valid cross-correlation per (b,c)

Sharding: pure data-parallel over B (16 batches per core).

Algorithm per core: depthwise conv has no shared operand for a dense
matmul, so we map each kernel tap (u,v) to a *diagonal* matmul:
  psum[c, :] += diag(z[:, u, v]) @ x[:, shifted window]
with the tap loop accumulating natively in PSUM. lhsT diag matrices are
precomputed host-side (bf16), the shifted rhs windows are pure access
patterns on the SBUF-resident x tile (no data movement).
"""

import numpy as np
import ml_dtypes

import concourse.bass as bass
import concourse.mybir as mybir
import concourse.tile as tile
from concourse import bacc
from concourse.bass_utils import run_bass_kernel_spmd

B, C = 128, 256
HX, WX = 31, 31
HZ, WZ = 7, 7
HO, WO = HX - HZ + 1, WX - WZ + 1  # 25, 25
NCORES = 8
BPC = B // NCORES         # batches per core = 16
Q = BPC * C               # (b,c) channels per core = 4096
G = Q // 128              # groups of 128 channels = 32
NX = HX * WX              # 961
NO = HO * WO              # 625
NT = HZ * WZ              # 49 taps
ROWS_A = 20               # output rows in psum chunk A (20*25=500 <= 512)
ROWS_B = HO - ROWS_A      # 5 rows (125 cols)

BF16 = ml_dtypes.bfloat16

_built = {}


def _ensure_ntff_hook():
    """Install the axon NTFF profiling hook if the container's antenv stub
    lacks it (needed only for trace=True local profiling runs)."""
    import contextlib
    import ctypes
    import sys
    import types

    try:
        from antenv.axon_hooks import get_axon_ntff_profile_hook  # noqa: F401

        return True
    except ImportError:
        pass
    so_path = "/opt/axon/libaxon_pjrt.so"
    try:
        lib = ctypes.CDLL(so_path)
    except OSError:
        return False
    if not hasattr(lib, "axon_start_nrt_profile"):
        return False
    lib.axon_start_nrt_profile.argtypes = [
        ctypes.POINTER(ctypes.c_int64),
        ctypes.c_size_t,
    ]
    lib.axon_start_nrt_profile.restype = ctypes.c_int64
    lib.axon_stop_nrt_profile.argtypes = [ctypes.c_char_p]
    lib.axon_stop_nrt_profile.restype = ctypes.c_int64

    @contextlib.contextmanager
    def _hook(output_dir, device_ids):
        import jax

        jax.devices()
        if device_ids:
            ids = (ctypes.c_int64 * len(device_ids))(*device_ids)
            rc = lib.axon_start_nrt_profile(ids, len(device_ids))
        else:
            rc = lib.axon_start_nrt_profile(None, 0)
        if rc != 0:
            raise RuntimeError(f"axon_start_nrt_profile rc={rc}")
        try:
            yield
        finally:
            n = lib.axon_stop_nrt_profile(str(output_dir).encode())
            print(f"profile: {n} file(s) written to {output_dir}", file=sys.stderr)

    state = {"hook": _hook}
    mod = types.ModuleType("antenv.axon_hooks")
    mod.get_axon_ntff_profile_hook = lambda: state["hook"]
    mod.set_axon_ntff_profile_hook = lambda h: state.update(hook=h)
    import antenv

    sys.modules["antenv.axon_hooks"] = mod
    antenv.axon_hooks = mod
    return True


def _build():
    """Build + compile the SPMD Bass program (cached per process)."""
    if "nc" in _built:
        return _built["nc"]

    nc = bacc.Bacc(
        "TRN2", target_bir_lowering=False, debug=False, num_devices=NCORES
    )
    x_d = nc.dram_tensor("x", [Q, NX], mybir.dt.bfloat16, kind="ExternalInput").ap()
    zd_d = nc.dram_tensor(
        "zd", [G, 128, NT, 128], mybir.dt.bfloat16, kind="ExternalInput"
    ).ap()
    out_d = nc.dram_tensor("out", [Q, NO], mybir.dt.float32, kind="ExternalOutput").ap()

    with tile.TileContext(nc) as tc:
        with (
            tc.tile_pool(name="xp", bufs=2) as xp,
            tc.tile_pool(name="zp", bufs=2) as zp,
            tc.tile_pool(name="op", bufs=2) as op,
            tc.tile_pool(name="psA", bufs=2, space="PSUM") as psA,
            tc.tile_pool(name="psB", bufs=2, space="PSUM") as psB,
        ):
            for g in range(G):
                x_sb = xp.tile([128, HX, WX], mybir.dt.bfloat16)
                zd_sb = zp.tile([128, NT, 128], mybir.dt.bfloat16)
                nc.sync.dma_start(out=x_sb, in_=x_d[g * 128 : (g + 1) * 128])
                nc.sync.dma_start(out=zd_sb, in_=zd_d[g])

                pA = psA.tile([128, ROWS_A * WO], mybir.dt.float32)
                pB = psB.tile([128, ROWS_B * WO], mybir.dt.float32)
                for t in range(NT):
                    u, v = divmod(t, WZ)
                    lhsT = zd_sb[:, t, :]
                    nc.tensor.matmul(
                        pA[:, :],
                        lhsT,
                        x_sb[:, u : u + ROWS_A, v : v + WO],
                        start=(t == 0),
                        stop=(t == NT - 1),
                    )
                    nc.tensor.matmul(
                        pB[:, :],
                        lhsT,
                        x_sb[:, ROWS_A + u : ROWS_A + u + ROWS_B, v : v + WO],
                        start=(t == 0),
                        stop=(t == NT - 1),
                    )

                out_sb = op.tile([128, NO], mybir.dt.float32)
                nc.vector.tensor_copy(out=out_sb[:, : ROWS_A * WO], in_=pA[:, :])
                nc.scalar.copy(out=out_sb[:, ROWS_A * WO :], in_=pB[:, :])
                nc.sync.dma_start(out=out_d[g * 128 : (g + 1) * 128], in_=out_sb)

    nc.compile()
    _built["nc"] = nc
    return nc


def _host_prep(z_f: np.ndarray, x_f: np.ndarray):
    """Shard + reformat inputs for the 8 cores."""
    x = np.ascontiguousarray(x_f, dtype=np.float32).reshape(B, C, NX)
    z = np.ascontiguousarray(z_f, dtype=np.float32).reshape(B, C, NT)
    in_maps = []
    p_idx = np.arange(128)
    for k in range(NCORES):
        xs = x[k * BPC : (k + 1) * BPC].reshape(Q, NX).astype(BF16)
        zs = z[k * BPC : (k + 1) * BPC].reshape(G, 128, NT).astype(BF16)
        zd = np.zeros((G, 128, NT, 128), dtype=BF16)
        # zd[g, p, t, p] = z[g*128+p, t]
        zd[:, p_idx, :, p_idx] = zs.transpose(1, 0, 2)
        in_maps.append({"x": xs, "zd": zd})
    return in_maps


def _run(z_f, x_f, trace=False, **spmd_kwargs):
    nc = _build()
    in_maps = _host_prep(z_f, x_f)
    if trace:
        _ensure_ntff_hook()
        # local profiling only — skip the artifact share upload
        import concourse.bass_utils as _bu

        _bu.upload_artifacts = lambda tmpdir: tmpdir
    res = run_bass_kernel_spmd(
        nc, in_maps, core_ids=list(range(NCORES)), trace=trace, **spmd_kwargs
    )
    outs = [np.asarray(r["out"], dtype=np.float32) for r in res.results]
    full = np.concatenate(outs, axis=0).reshape(B, C, HO, WO)
    return full, res


def kernel(z_f: np.ndarray, x_f: np.ndarray) -> np.ndarray:
    full, _ = _run(z_f, x_f, trace=False)
    return full
